# revision 14
# baseline (speedup 1.0000x reference)
"""Trainium2 Bass kernel for nn_ColorTransform: per-pixel degree-3 polynomial
color transform  y[b,c,h,w] = bias[c] + sum_f weight[f,c] * mono_f(x[b,:,h,w]).

Strategy (pure data parallel over batch across 8 cores; identical SPMD program):

The 3->19->3 per-pixel map is a degree-<=3 polynomial in the 3 channels.
Cubes and squares of R affine forms L_i = a_i.x + b_i represent it:

    y_c = sum_i cq[i,c] * L_i^3 + cs[i,c] * L_i^2

with coefficients (and for R<10, the forms too) solved at runtime on the host
from (weight, bias).

On-chip pipeline: pixels are packed GROUPS-per-chunk x pixel-columns on the
partition dim (R form-rows per group). Per compute-chunk of NCMP columns:
  PE  M1  -> P1 = lhsT1^T @ X (block-diag forms)     [G*R, NCMP] PSUM
  ACT     -> S = Square(P1)  (fp32r)                 SBUF
  DVE     -> Q = S * P1      (cube, fp32r)           SBUF
  PE  M2  -> P2 = Wq^T @ Q + Ws^T @ S (PSUM accum)   [G*3, NCMP] PSUM
  ACT/DVE -> O copy-out (split by columns)           SBUF
DMA granularity is ND columns (ND/NCMP compute chunks per DMA) with a v-major
row layout so each (batch, direction) is ONE 3-dim DMA.
"""
import numpy as np
from itertools import product as _product
from math import factorial as _factorial

import concourse.bass as bass
import concourse.tile as tile
from concourse import bacc, mybir
from concourse.bass_utils import run_bass_kernel_spmd

# ---------------------------------------------------------------- constants
B, C, H, W = 16, 3, 512, 512
HW = H * W
NCORES = 8
BPC = B // NCORES          # batches per core = 2
R = 10                     # affine forms per group
GPB = 6                    # groups per batch per chunk
NG = BPC * GPB             # total groups per chunk
ND = 4096                  # DMA columns per chunk
NCMP = 1024                # compute columns per sub-chunk
SPLIT = ND // NCMP
FULL_CHUNKS = HW // (GPB * ND)            # 10
TAIL_PX = HW - FULL_CHUNKS * GPB * ND     # 16384 per plane
TAIL_GPB = TAIL_PX // ND                  # 4 (0 => no tail)
TAIL_NG = BPC * TAIL_GPB
ACT_FRAC = 0.61            # out-copy column split fraction done on ACT

# optimized generic affine forms (see form_opt.py)
AV = np.array([
    [ 0.37934126,  0.23092419,  0.89597669],
    [-0.11446939,  0.06385343,  0.99137253],
    [ 0.21945084, -0.83239185,  0.50888617],
    [-0.6455188 , -0.57861811,  0.49850432],
    [-0.02451489,  0.30102502, -0.95330108],
    [ 0.09930513,  0.99370851, -0.0517869 ],
    [-0.56684164,  0.6902054 ,  0.44978558],
    [-0.71569315, -0.69804976,  0.02257986],
    [ 0.94752367, -0.15834609, -0.27771463],
    [ 0.99764591, -0.01721242,  0.06638047],
])
BV = np.array([-0.58237884,  0.03295331,  0.14354757,  0.34220693,  0.78767153,
               -0.00392558, -0.31987566,  1.06484995, -0.16575755,  0.0089387 ])

MONOMIALS = [
    (1,0,0),(0,1,0),(0,0,1),
    (2,0,0),(1,1,0),(1,0,1),(0,2,0),(0,1,1),(0,0,2),
    (3,0,0),(2,1,0),(2,0,1),(1,2,0),(1,1,1),(1,0,2),(0,3,0),(0,2,1),(0,1,2),(0,0,3),
]
ALL_MONO = [(0,0,0)] + MONOMIALS


def _expand(a, b, power):
    coeffs = {}
    for ks in _product(range(power+1), repeat=4):
        if sum(ks) != power:
            continue
        k0, k1, k2, kb = ks
        mult = _factorial(power)/(_factorial(k0)*_factorial(k1)*_factorial(k2)*_factorial(kb))
        coeffs[(k0,k1,k2)] = coeffs.get((k0,k1,k2), 0.0) + \
            mult * a[0]**k0 * a[1]**k1 * a[2]**k2 * b**kb
    return np.array([coeffs.get(m, 0.0) for m in ALL_MONO])


def _solve_coeffs(weight, bias):
    """-> av [R,3], bv [R], cq [R,3], cs [R,3] (float32/64 mix)."""
    A = np.stack([_expand(AV[i], BV[i], 3) for i in range(R)] +
                 [_expand(AV[i], BV[i], 2) for i in range(R)], axis=1)
    T = np.zeros((20, 3))
    T[0] = np.asarray(bias, np.float64)
    T[1:] = np.asarray(weight, np.float64)
    Cf = np.linalg.lstsq(A, T, rcond=None)[0]
    return AV, BV, Cf[:R].astype(np.float32), Cf[R:].astype(np.float32)


# v-major row maps -----------------------------------------------------------
# X tile rows: 0 = ones; 1 + b*(3*gpb) + v*gpb + g
# P1 rows (forms): b*(R*gpb) + i*gpb + g      (i-major within batch!)
# P2/O rows:       b*(3*gpb) + c*gpb + g

def _lhs1(av, bv, gpb):
    KX = 3 * gpb * BPC + 1
    m = np.zeros((KX, R * gpb * BPC), np.float32)
    for b in range(BPC):
        for g in range(gpb):
            for i in range(R):
                col = b*R*gpb + i*gpb + g
                m[0, col] = bv[i]
                for v in range(C):
                    m[1 + b*3*gpb + v*gpb + g, col] = av[i, v]
    return m.astype(np.float16)


def _lhs2(coeff, gpb):
    m = np.zeros((R * gpb * BPC, 3 * gpb * BPC), np.float32)
    for b in range(BPC):
        for g in range(gpb):
            for i in range(R):
                for c in range(C):
                    m[b*R*gpb + i*gpb + g, b*3*gpb + c*gpb + g] = coeff[i, c]
    return m


# ---------------------------------------------------------------- bass build
_NC_CACHE = {}


def build_nc(reps=1):
    if reps in _NC_CACHE:
        return _NC_CACHE[reps]
    f32, f16, f32r = mybir.dt.float32, mybir.dt.float16, mybir.dt.float32r
    nc = bacc.Bacc("TRN2", target_bir_lowering=False, debug=False, num_devices=NCORES)

    KX = 3 * GPB * BPC + 1
    RW = R * GPB * BPC
    OW = 3 * GPB * BPC
    KXt = 3 * TAIL_GPB * BPC + 1
    RWt = R * TAIL_GPB * BPC
    OWt = 3 * TAIL_GPB * BPC

    xs = nc.dram_tensor("xs", [BPC, C, HW], f16, kind="ExternalInput")
    wm1 = nc.dram_tensor("wm1", [KX, RW], f16, kind="ExternalInput")
    w2q = nc.dram_tensor("w2q", [RW, OW], f32, kind="ExternalInput")
    w2s = nc.dram_tensor("w2s", [RW, OW], f32, kind="ExternalInput")
    y = nc.dram_tensor("y", [BPC, C, HW], f32, kind="ExternalOutput")
    if TAIL_GPB:
        wm1t = nc.dram_tensor("wm1t", [KXt, RWt], f16, kind="ExternalInput")
        w2qt = nc.dram_tensor("w2qt", [RWt, OWt], f32, kind="ExternalInput")
        w2st = nc.dram_tensor("w2st", [RWt, OWt], f32, kind="ExternalInput")

    with tile.TileContext(nc) as tc:
        with (
            tc.tile_pool(name="wpool", bufs=1) as wpool,
            tc.tile_pool(name="xpool", bufs=2) as xpool,
            tc.tile_pool(name="spool", bufs=3) as spool,
            tc.tile_pool(name="qpool", bufs=3) as qpool,
            tc.tile_pool(name="opool", bufs=2) as opool,
            tc.tile_pool(name="p1pool", bufs=2, space="PSUM") as p1pool,
            tc.tile_pool(name="p2pool", bufs=2, space="PSUM") as p2pool,
        ):
            def load_w(name, dram, shape, dt_, round_to=None):
                t = wpool.tile(shape, dt_, tag=name)
                nc.sync.dma_start(t[:], dram[:])
                if round_to is None:
                    return t
                tr = wpool.tile(shape, round_to, tag=name + "r")
                nc.vector.tensor_copy(tr[:], t[:])
                return tr

            wm1_sb = load_w("wm1", wm1, [KX, RW], f16)
            w2q_r = load_w("w2q", w2q, [RW, OW], f32, f32r)
            w2s_r = load_w("w2s", w2s, [RW, OW], f32, f32r)
            if TAIL_GPB:
                wm1t_sb = load_w("wm1t", wm1t, [KXt, RWt], f16)
                w2qt_r = load_w("w2qt", w2qt, [RWt, OWt], f32, f32r)
                w2st_r = load_w("w2st", w2st, [RWt, OWt], f32, f32r)

            for _ in range(2):
                xt0 = xpool.tile([KX, ND], f16, tag="X")
                nc.gpsimd.memset(xt0[0:1, :], 1.0)

            def chunk(lo, gpb, m1w, qw, sw, tagsuf):
                kx = 3 * gpb * BPC + 1
                rw = R * gpb * BPC
                ow = 3 * gpb * BPC
                xt = xpool.tile([kx, ND], f16, tag="X" + tagsuf)
                if tagsuf:
                    nc.gpsimd.memset(xt[0:1, :], 1.0)
                for b in range(BPC):
                    nc.sync.dma_start(
                        xt[1+b*3*gpb:1+(b+1)*3*gpb],
                        xs[b, :, lo:lo+gpb*ND].rearrange("v (g n) -> v g n", n=ND))
                o = opool.tile([ow, ND], f32, tag="O" + tagsuf)
                for q in range(SPLIT):
                    cl, ch = q * NCMP, (q + 1) * NCMP
                    p1 = p1pool.tile([rw, NCMP], f32, tag="P1")
                    for h in range(NCMP // 512):
                        nc.tensor.matmul(p1[:, h*512:(h+1)*512], m1w[:],
                                         xt[:, cl+h*512:cl+(h+1)*512],
                                         start=True, stop=True)
                    s = spool.tile([rw, NCMP], f32r, tag="S")
                    nc.scalar.square(s[:], p1[:])
                    qq = qpool.tile([rw, NCMP], f32r, tag="Q")
                    nc.vector.tensor_mul(qq[:], s[:], p1[:])
                    p2 = p2pool.tile([ow, NCMP], f32, tag="P2")
                    for h in range(NCMP // 512):
                        hl, hh = h*512, (h+1)*512
                        nc.tensor.matmul(p2[:, hl:hh], qw[:], qq[:, hl:hh],
                                         start=True, stop=False)
                        nc.tensor.matmul(p2[:, hl:hh], sw[:], s[:, hl:hh],
                                         start=False, stop=True)
                    asp = (int(NCMP * ACT_FRAC) // 2) * 2
                    nc.scalar.copy(o[:, cl:cl+asp], p2[:, 0:asp])
                    nc.vector.tensor_copy(o[:, cl+asp:ch], p2[:, asp:NCMP])
                for b in range(BPC):
                    nc.sync.dma_start(
                        y[b, :, lo:lo+gpb*ND].rearrange("c (g n) -> c g n", n=ND),
                        o[b*3*gpb:(b+1)*3*gpb])

            def body():
                for k in range(FULL_CHUNKS):
                    chunk(k * GPB * ND, GPB, wm1_sb, w2q_r, w2s_r, "")
                if TAIL_GPB:
                    chunk(FULL_CHUNKS * GPB * ND, TAIL_GPB,
                          wm1t_sb, w2qt_r, w2st_r, "T")

            if reps == 1:
                body()
            else:
                hint = (mybir.EngineType.PE, mybir.EngineType.Activation,
                        mybir.EngineType.DVE, mybir.EngineType.SP)
                with tc.For_i(0, reps, 1, hint_engines=hint):
                    body()

    nc.compile()
    _NC_CACHE[reps] = nc
    return nc


def make_in_maps(x, weight, bias):
    av, bv, cq, cs = _solve_coeffs(weight, bias)
    shared = {
        "wm1": _lhs1(av, bv, GPB),
        "w2q": _lhs2(cq, GPB), "w2s": _lhs2(cs, GPB),
    }
    if TAIL_GPB:
        shared.update({
            "wm1t": _lhs1(av, bv, TAIL_GPB),
            "w2qt": _lhs2(cq, TAIL_GPB), "w2st": _lhs2(cs, TAIL_GPB),
        })
    x = np.ascontiguousarray(np.asarray(x, np.float16)).reshape(B, C, HW)
    return [dict(shared, xs=x[i*BPC:(i+1)*BPC]) for i in range(NCORES)]


def kernel(x, weight, bias, degree=3, **_unused):
    assert int(degree) == 3, "kernel specialized for degree=3"
    nc = build_nc(reps=1)
    in_maps = make_in_maps(x, weight, bias)
    res = run_bass_kernel_spmd(nc, in_maps, core_ids=list(range(NCORES)))
    out = np.empty((B, C, HW), np.float32)
    for i in range(NCORES):
        out[i*BPC:(i+1)*BPC] = res.results[i]["y"]
    return out.reshape(B, C, H, W)


if __name__ == "__main__":
    rng = np.random.default_rng(0)
    x = rng.uniform(0, 1, size=(B, C, H, W)).astype(np.float32)
    weight = rng.normal(size=(19, 3)).astype(np.float32)
    bias = rng.normal(size=(3,)).astype(np.float32)
    got = kernel(x, weight, bias, 3)
    print("ran; out shape", got.shape)


# revision 24
# speedup vs baseline: 168.4974x; 168.4974x over previous
"""Trainium2 Bass kernel for nn_ColorTransform: per-pixel degree-3 polynomial
color transform  y[b,c,h,w] = bias[c] + sum_f weight[f,c] * mono_f(x[b,:,h,w]).

Strategy (pure data parallel over batch across 8 cores; identical SPMD program):

The 3->19->3 per-pixel map is a degree-<=3 polynomial in the 3 channels.
Cubes and squares of R affine forms L_i = a_i.x + b_i represent it:

    y_c = sum_i cq[i,c] * L_i^3 + cs[i,c] * L_i^2

with coefficients (and for R<10, the forms too) solved at runtime on the host
from (weight, bias).

On-chip pipeline: pixels are packed GROUPS-per-chunk x pixel-columns on the
partition dim (R form-rows per group). Per compute-chunk of NCMP columns:
  PE  M1  -> P1 = lhsT1^T @ X (block-diag forms)     [G*R, NCMP] PSUM
  ACT     -> S = Square(P1)  (fp32r)                 SBUF
  DVE     -> Q = S * P1      (cube, fp32r)           SBUF
  PE  M2  -> P2 = Wq^T @ Q + Ws^T @ S (PSUM accum)   [G*3, NCMP] PSUM
  ACT/DVE -> O copy-out (split by columns)           SBUF
DMA granularity is ND columns (ND/NCMP compute chunks per DMA) with a v-major
row layout so each (batch, direction) is ONE 3-dim DMA.
"""
import numpy as np
from itertools import product as _product
from math import factorial as _factorial

import concourse.bass as bass
import concourse.tile as tile
from concourse import bacc, mybir
from concourse.bass_utils import run_bass_kernel_spmd

# ---------------------------------------------------------------- constants
B, C, H, W = 16, 3, 512, 512
HW = H * W
NCORES = 8
BPC = B // NCORES          # batches per core = 2
R = 10                     # affine forms per group
GPB = 6                    # groups per batch per chunk
NG = BPC * GPB             # total groups per chunk
ND = 4096                  # DMA columns per chunk
NCMP = 1024                # compute columns per sub-chunk
SPLIT = ND // NCMP
FULL_CHUNKS = HW // (GPB * ND)            # 10
TAIL_PX = HW - FULL_CHUNKS * GPB * ND     # 16384 per plane
TAIL_GPB = TAIL_PX // ND                  # 4 (0 => no tail)
TAIL_NG = BPC * TAIL_GPB
ACT_FRAC = 0.61            # out-copy column split fraction done on ACT
FEAT_F16 = False           # True: features + M2 weights in fp16 instead of fp32r

# optimized generic affine forms (see form_opt.py)
AV = np.array([
    [ 0.37934126,  0.23092419,  0.89597669],
    [-0.11446939,  0.06385343,  0.99137253],
    [ 0.21945084, -0.83239185,  0.50888617],
    [-0.6455188 , -0.57861811,  0.49850432],
    [-0.02451489,  0.30102502, -0.95330108],
    [ 0.09930513,  0.99370851, -0.0517869 ],
    [-0.56684164,  0.6902054 ,  0.44978558],
    [-0.71569315, -0.69804976,  0.02257986],
    [ 0.94752367, -0.15834609, -0.27771463],
    [ 0.99764591, -0.01721242,  0.06638047],
])
BV = np.array([-0.58237884,  0.03295331,  0.14354757,  0.34220693,  0.78767153,
               -0.00392558, -0.31987566,  1.06484995, -0.16575755,  0.0089387 ])

MONOMIALS = [
    (1,0,0),(0,1,0),(0,0,1),
    (2,0,0),(1,1,0),(1,0,1),(0,2,0),(0,1,1),(0,0,2),
    (3,0,0),(2,1,0),(2,0,1),(1,2,0),(1,1,1),(1,0,2),(0,3,0),(0,2,1),(0,1,2),(0,0,3),
]
ALL_MONO = [(0,0,0)] + MONOMIALS


def _expand(a, b, power):
    coeffs = {}
    for ks in _product(range(power+1), repeat=4):
        if sum(ks) != power:
            continue
        k0, k1, k2, kb = ks
        mult = _factorial(power)/(_factorial(k0)*_factorial(k1)*_factorial(k2)*_factorial(kb))
        coeffs[(k0,k1,k2)] = coeffs.get((k0,k1,k2), 0.0) + \
            mult * a[0]**k0 * a[1]**k1 * a[2]**k2 * b**kb
    return np.array([coeffs.get(m, 0.0) for m in ALL_MONO])


def _solve_coeffs(weight, bias):
    """-> av [R,3], bv [R], cq [R,3], cs [R,3] (float32/64 mix)."""
    A = np.stack([_expand(AV[i], BV[i], 3) for i in range(R)] +
                 [_expand(AV[i], BV[i], 2) for i in range(R)], axis=1)
    T = np.zeros((20, 3))
    T[0] = np.asarray(bias, np.float64)
    T[1:] = np.asarray(weight, np.float64)
    Cf = np.linalg.lstsq(A, T, rcond=None)[0]
    return AV, BV, Cf[:R].astype(np.float32), Cf[R:].astype(np.float32)


# v-major row maps -----------------------------------------------------------
# X tile rows: 0 = ones; 1 + b*(3*gpb) + v*gpb + g
# P1 rows (forms): b*(R*gpb) + i*gpb + g      (i-major within batch!)
# P2/O rows:       b*(3*gpb) + c*gpb + g

def _lhs1(av, bv, gpb):
    KX = 3 * gpb * BPC + 1
    m = np.zeros((KX, R * gpb * BPC), np.float32)
    for b in range(BPC):
        for g in range(gpb):
            for i in range(R):
                col = b*R*gpb + i*gpb + g
                m[0, col] = bv[i]
                for v in range(C):
                    m[1 + b*3*gpb + v*gpb + g, col] = av[i, v]
    return m.astype(np.float16)


def _lhs2(coeff, gpb):
    m = np.zeros((R * gpb * BPC, 3 * gpb * BPC), np.float32)
    for b in range(BPC):
        for g in range(gpb):
            for i in range(R):
                for c in range(C):
                    m[b*R*gpb + i*gpb + g, b*3*gpb + c*gpb + g] = coeff[i, c]
    return m


# ---------------------------------------------------------------- bass build
_NC_CACHE = {}


def build_nc(reps=1, chunks=None):
    key = (reps, chunks)
    if key in _NC_CACHE:
        return _NC_CACHE[key]
    f32, f16, f32r = mybir.dt.float32, mybir.dt.float16, mybir.dt.float32r
    nc = bacc.Bacc("TRN2", target_bir_lowering=False, debug=False, num_devices=NCORES)

    KX = 3 * GPB * BPC + 1
    RW = R * GPB * BPC
    OW = 3 * GPB * BPC
    KXt = 3 * TAIL_GPB * BPC + 1
    RWt = R * TAIL_GPB * BPC
    OWt = 3 * TAIL_GPB * BPC

    xs = nc.dram_tensor("xs", [BPC, C, HW], f16, kind="ExternalInput")
    wm1 = nc.dram_tensor("wm1", [KX, RW], f16, kind="ExternalInput")
    w2q = nc.dram_tensor("w2q", [RW, OW], f32, kind="ExternalInput")
    w2s = nc.dram_tensor("w2s", [RW, OW], f32, kind="ExternalInput")
    y = nc.dram_tensor("y", [BPC, C, HW], f32, kind="ExternalOutput")
    if TAIL_GPB:
        wm1t = nc.dram_tensor("wm1t", [KXt, RWt], f16, kind="ExternalInput")
        w2qt = nc.dram_tensor("w2qt", [RWt, OWt], f32, kind="ExternalInput")
        w2st = nc.dram_tensor("w2st", [RWt, OWt], f32, kind="ExternalInput")

    with tile.TileContext(nc) as tc:
        with (
            tc.tile_pool(name="wpool", bufs=1) as wpool,
            tc.tile_pool(name="xpool", bufs=2) as xpool,
            tc.tile_pool(name="spool", bufs=3) as spool,
            tc.tile_pool(name="qpool", bufs=3) as qpool,
            tc.tile_pool(name="opool", bufs=2) as opool,
            tc.tile_pool(name="p1pool", bufs=2, space="PSUM") as p1pool,
            tc.tile_pool(name="p2pool", bufs=2, space="PSUM") as p2pool,
        ):
            def load_w(name, dram, shape, dt_, round_to=None):
                t = wpool.tile(shape, dt_, tag=name)
                nc.sync.dma_start(t[:], dram[:])
                if round_to is None:
                    return t
                tr = wpool.tile(shape, round_to, tag=name + "r")
                nc.vector.tensor_copy(tr[:], t[:])
                return tr

            fdt = f16 if FEAT_F16 else f32r
            wm1_sb = load_w("wm1", wm1, [KX, RW], f16)
            w2q_r = load_w("w2q", w2q, [RW, OW], f32, fdt)
            w2s_r = load_w("w2s", w2s, [RW, OW], f32, fdt)
            if TAIL_GPB:
                wm1t_sb = load_w("wm1t", wm1t, [KXt, RWt], f16)
                w2qt_r = load_w("w2qt", w2qt, [RWt, OWt], f32, fdt)
                w2st_r = load_w("w2st", w2st, [RWt, OWt], f32, fdt)

            for _ in range(2):
                xt0 = xpool.tile([KX, ND], f16, tag="X")
                nc.gpsimd.memset(xt0[0:1, :], 1.0)

            # Software pipeline: stage1 (DMA-in + M1) of compute-chunk j is
            # emitted before stage2 (Square/cube/M2/copy-out + DMA-out) of
            # chunk j-1, so no engine queue head-of-line blocks on the
            # M1 -> SQ -> TT -> M2 dependency chain.
            asp = (int(NCMP * ACT_FRAC) // 2) * 2

            def stage1(lo, gpb, m1w, tagsuf, q, xt_state):
                kx = 3 * gpb * BPC + 1
                rw = R * gpb * BPC
                if q == 0:
                    xt = xpool.tile([kx, ND], f16, tag="X" + tagsuf, name="xt" + tagsuf)
                    if tagsuf:
                        nc.gpsimd.memset(xt[0:1, :], 1.0)
                    for b in range(BPC):
                        nc.sync.dma_start(
                            xt[1+b*3*gpb:1+(b+1)*3*gpb],
                            xs[b, :, lo:lo+gpb*ND].rearrange("v (g n) -> v g n", n=ND))
                    xt_state[tagsuf] = xt
                xt = xt_state[tagsuf]
                cl = q * NCMP
                p1 = p1pool.tile([rw, NCMP], f32, tag="P1")
                for h in range(NCMP // 512):
                    nc.tensor.matmul(p1[:, h*512:(h+1)*512], m1w[:],
                                     xt[:, cl+h*512:cl+(h+1)*512],
                                     start=True, stop=True)
                return p1

            def stage2(gpb, p1):
                rw = R * gpb * BPC
                s = spool.tile([rw, NCMP], f16 if FEAT_F16 else f32r, tag="S")
                nc.scalar.square(s[:], p1[:])
                qq = qpool.tile([rw, NCMP], f16 if FEAT_F16 else f32r, tag="Q")
                nc.vector.tensor_mul(qq[:], s[:], p1[:])
                return s, qq

            def stage3(lo, gpb, qw, sw, tagsuf, q, s, qq, o_state):
                rw = R * gpb * BPC
                ow = 3 * gpb * BPC
                if q == 0:
                    o_state[tagsuf] = opool.tile([ow, ND], f32, tag="O" + tagsuf, name="o" + tagsuf)
                o = o_state[tagsuf]
                cl, ch = q * NCMP, (q + 1) * NCMP
                p2 = p2pool.tile([ow, NCMP], f32, tag="P2")
                for h in range(NCMP // 512):
                    hl, hh = h*512, (h+1)*512
                    nc.tensor.matmul(p2[:, hl:hh], qw[:], qq[:, hl:hh],
                                     start=True, stop=False)
                    nc.tensor.matmul(p2[:, hl:hh], sw[:], s[:, hl:hh],
                                     start=False, stop=True)
                nc.scalar.copy(o[:, cl:cl+asp], p2[:, 0:asp])
                nc.vector.tensor_copy(o[:, cl+asp:ch], p2[:, asp:NCMP])
                if q == SPLIT - 1:
                    for b in range(BPC):
                        nc.sync.dma_start(
                            y[b, :, lo:lo+gpb*ND].rearrange("c (g n) -> c g n", n=ND),
                            o[b*3*gpb:(b+1)*3*gpb])

            def body():
                nfull = FULL_CHUNKS if chunks is None else chunks
                work = [(k * GPB * ND, GPB, wm1_sb, w2q_r, w2s_r, "")
                        for k in range(nfull)]
                if TAIL_GPB and chunks is None:
                    work.append((FULL_CHUNKS * GPB * ND, TAIL_GPB,
                                 wm1t_sb, w2qt_r, w2st_r, "T"))
                units = [(lo, gpb, m1w, qw, sw, tagsuf, q)
                         for (lo, gpb, m1w, qw, sw, tagsuf) in work
                         for q in range(SPLIT)]
                xt_state, o_state = {}, {}
                q2 = []   # awaiting stage2, FIFO
                q3 = []   # awaiting stage3, FIFO
                D2, D3 = 1, 1   # stage separation depths (units)

                def pump(force=False):
                    if q2 and (force or len(q2) > D2 - 1):
                        (l2, g2, q2w, s2w, t2, qu2, p12) = q2.pop(0)
                        s_t, qq_t = stage2(g2, p12)
                        q3.append((l2, g2, q2w, s2w, t2, qu2, s_t, qq_t, o_state))
                    if q3 and (force or len(q3) > D3 - 1):
                        stage3(*q3.pop(0))

                for (lo, gpb, m1w, qw, sw, tagsuf, q) in units:
                    p1 = stage1(lo, gpb, m1w, tagsuf, q, xt_state)
                    q2.append((lo, gpb, qw, sw, tagsuf, q, p1))
                    pump()
                while q2 or q3:
                    pump(force=True)

            if reps == 1:
                body()
            else:
                hint = (mybir.EngineType.PE, mybir.EngineType.Activation,
                        mybir.EngineType.DVE, mybir.EngineType.SP)
                with tc.For_i(0, reps, 1, hint_engines=hint):
                    body()

    nc.compile()
    _NC_CACHE[key] = nc
    return nc


def make_in_maps(x, weight, bias):
    av, bv, cq, cs = _solve_coeffs(weight, bias)
    shared = {
        "wm1": _lhs1(av, bv, GPB),
        "w2q": _lhs2(cq, GPB), "w2s": _lhs2(cs, GPB),
    }
    if TAIL_GPB:
        shared.update({
            "wm1t": _lhs1(av, bv, TAIL_GPB),
            "w2qt": _lhs2(cq, TAIL_GPB), "w2st": _lhs2(cs, TAIL_GPB),
        })
    x = np.ascontiguousarray(np.asarray(x, np.float16)).reshape(B, C, HW)
    return [dict(shared, xs=x[i*BPC:(i+1)*BPC]) for i in range(NCORES)]


def kernel(x, weight, bias, degree=3, **_unused):
    assert int(degree) == 3, "kernel specialized for degree=3"
    nc = build_nc(reps=1)
    in_maps = make_in_maps(x, weight, bias)
    res = run_bass_kernel_spmd(nc, in_maps, core_ids=list(range(NCORES)))
    out = np.empty((B, C, HW), np.float32)
    for i in range(NCORES):
        out[i*BPC:(i+1)*BPC] = res.results[i]["y"]
    return out.reshape(B, C, H, W)


if __name__ == "__main__":
    rng = np.random.default_rng(0)
    x = rng.uniform(0, 1, size=(B, C, H, W)).astype(np.float32)
    weight = rng.normal(size=(19, 3)).astype(np.float32)
    bias = rng.normal(size=(3,)).astype(np.float32)
    got = kernel(x, weight, bias, 3)
    print("ran; out shape", got.shape)


# revision 25
# speedup vs baseline: 218.1877x; 1.2949x over previous
"""Trainium2 Bass kernel for nn_ColorTransform: per-pixel degree-3 polynomial
color transform  y[b,c,h,w] = bias[c] + sum_f weight[f,c] * mono_f(x[b,:,h,w]).

Strategy (pure data parallel over batch across 8 cores; identical SPMD program):

The 3->19->3 per-pixel map is a degree-<=3 polynomial in the 3 channels.
Cubes and squares of R affine forms L_i = a_i.x + b_i represent it:

    y_c = sum_i cq[i,c] * L_i^3 + cs[i,c] * L_i^2

with coefficients (and for R<10, the forms too) solved at runtime on the host
from (weight, bias).

On-chip pipeline: pixels are packed GROUPS-per-chunk x pixel-columns on the
partition dim (R form-rows per group). Per compute-chunk of NCMP columns:
  PE  M1  -> P1 = lhsT1^T @ X (block-diag forms)     [G*R, NCMP] PSUM
  ACT     -> S = Square(P1)  (fp32r)                 SBUF
  DVE     -> Q = S * P1      (cube, fp32r)           SBUF
  PE  M2  -> P2 = Wq^T @ Q + Ws^T @ S (PSUM accum)   [G*3, NCMP] PSUM
  ACT/DVE -> O copy-out (split by columns)           SBUF
DMA granularity is ND columns (ND/NCMP compute chunks per DMA) with a v-major
row layout so each (batch, direction) is ONE 3-dim DMA.
"""
import numpy as np
from itertools import product as _product
from math import factorial as _factorial

import concourse.bass as bass
import concourse.tile as tile
from concourse import bacc, mybir
from concourse.bass_utils import run_bass_kernel_spmd

# ---------------------------------------------------------------- constants
B, C, H, W = 16, 3, 512, 512
HW = H * W
NCORES = 8
BPC = B // NCORES          # batches per core = 2
R = 10                     # affine forms per group
GPB = 6                    # groups per batch per chunk
NG = BPC * GPB             # total groups per chunk
ND = 4096                  # DMA columns per chunk
NCMP = 1024                # compute columns per sub-chunk
SPLIT = ND // NCMP
FULL_CHUNKS = HW // (GPB * ND)            # 10
TAIL_PX = HW - FULL_CHUNKS * GPB * ND     # 16384 per plane
TAIL_GPB = TAIL_PX // ND                  # 4 (0 => no tail)
TAIL_NG = BPC * TAIL_GPB
ACT_FRAC = 0.61            # out-copy column split fraction done on ACT
FEAT_F16 = False           # True: features + M2 weights in fp16 instead of fp32r

# optimized generic affine forms (see form_opt.py)
AV = np.array([
    [ 0.37934126,  0.23092419,  0.89597669],
    [-0.11446939,  0.06385343,  0.99137253],
    [ 0.21945084, -0.83239185,  0.50888617],
    [-0.6455188 , -0.57861811,  0.49850432],
    [-0.02451489,  0.30102502, -0.95330108],
    [ 0.09930513,  0.99370851, -0.0517869 ],
    [-0.56684164,  0.6902054 ,  0.44978558],
    [-0.71569315, -0.69804976,  0.02257986],
    [ 0.94752367, -0.15834609, -0.27771463],
    [ 0.99764591, -0.01721242,  0.06638047],
])
BV = np.array([-0.58237884,  0.03295331,  0.14354757,  0.34220693,  0.78767153,
               -0.00392558, -0.31987566,  1.06484995, -0.16575755,  0.0089387 ])

MONOMIALS = [
    (1,0,0),(0,1,0),(0,0,1),
    (2,0,0),(1,1,0),(1,0,1),(0,2,0),(0,1,1),(0,0,2),
    (3,0,0),(2,1,0),(2,0,1),(1,2,0),(1,1,1),(1,0,2),(0,3,0),(0,2,1),(0,1,2),(0,0,3),
]
ALL_MONO = [(0,0,0)] + MONOMIALS


def _expand(a, b, power):
    coeffs = {}
    for ks in _product(range(power+1), repeat=4):
        if sum(ks) != power:
            continue
        k0, k1, k2, kb = ks
        mult = _factorial(power)/(_factorial(k0)*_factorial(k1)*_factorial(k2)*_factorial(kb))
        coeffs[(k0,k1,k2)] = coeffs.get((k0,k1,k2), 0.0) + \
            mult * a[0]**k0 * a[1]**k1 * a[2]**k2 * b**kb
    return np.array([coeffs.get(m, 0.0) for m in ALL_MONO])


def _solve_coeffs(weight, bias):
    """-> av [R,3], bv [R], cq [R,3], cs [R,3] (float32/64 mix)."""
    A = np.stack([_expand(AV[i], BV[i], 3) for i in range(R)] +
                 [_expand(AV[i], BV[i], 2) for i in range(R)], axis=1)
    T = np.zeros((20, 3))
    T[0] = np.asarray(bias, np.float64)
    T[1:] = np.asarray(weight, np.float64)
    Cf = np.linalg.lstsq(A, T, rcond=None)[0]
    return AV, BV, Cf[:R].astype(np.float32), Cf[R:].astype(np.float32)


# v-major row maps -----------------------------------------------------------
# X tile rows: 0 = ones; 1 + b*(3*gpb) + v*gpb + g
# P1 rows (forms): b*(R*gpb) + i*gpb + g      (i-major within batch!)
# P2/O rows:       b*(3*gpb) + c*gpb + g

def _lhs1(av, bv, gpb):
    KX = 3 * gpb * BPC + 1
    m = np.zeros((KX, R * gpb * BPC), np.float32)
    for b in range(BPC):
        for g in range(gpb):
            for i in range(R):
                col = b*R*gpb + i*gpb + g
                m[0, col] = bv[i]
                for v in range(C):
                    m[1 + b*3*gpb + v*gpb + g, col] = av[i, v]
    return m.astype(np.float16)


def _lhs2(coeff, gpb):
    m = np.zeros((R * gpb * BPC, 3 * gpb * BPC), np.float32)
    for b in range(BPC):
        for g in range(gpb):
            for i in range(R):
                for c in range(C):
                    m[b*R*gpb + i*gpb + g, b*3*gpb + c*gpb + g] = coeff[i, c]
    return m


# ---------------------------------------------------------------- bass build
_NC_CACHE = {}


def build_nc(reps=1, chunks=None):
    key = (reps, chunks)
    if key in _NC_CACHE:
        return _NC_CACHE[key]
    f32, f16, f32r = mybir.dt.float32, mybir.dt.float16, mybir.dt.float32r
    nc = bacc.Bacc("TRN2", target_bir_lowering=False, debug=False, num_devices=NCORES)

    KX = 3 * GPB * BPC + 1
    RW = R * GPB * BPC
    OW = 3 * GPB * BPC
    KXt = 3 * TAIL_GPB * BPC + 1
    RWt = R * TAIL_GPB * BPC
    OWt = 3 * TAIL_GPB * BPC

    xs = nc.dram_tensor("xs", [BPC, C, HW], f16, kind="ExternalInput")
    wm1 = nc.dram_tensor("wm1", [KX, RW], f16, kind="ExternalInput")
    w2q = nc.dram_tensor("w2q", [RW, OW], f32, kind="ExternalInput")
    w2s = nc.dram_tensor("w2s", [RW, OW], f32, kind="ExternalInput")
    y = nc.dram_tensor("y", [BPC, C, HW], f32, kind="ExternalOutput")
    if TAIL_GPB:
        wm1t = nc.dram_tensor("wm1t", [KXt, RWt], f16, kind="ExternalInput")
        w2qt = nc.dram_tensor("w2qt", [RWt, OWt], f32, kind="ExternalInput")
        w2st = nc.dram_tensor("w2st", [RWt, OWt], f32, kind="ExternalInput")

    with tile.TileContext(nc) as tc:
        with (
            tc.tile_pool(name="wpool", bufs=1) as wpool,
            tc.tile_pool(name="xpool", bufs=2) as xpool,
            tc.tile_pool(name="spool", bufs=3) as spool,
            tc.tile_pool(name="qpool", bufs=3) as qpool,
            tc.tile_pool(name="opool", bufs=2) as opool,
            tc.tile_pool(name="p1pool", bufs=2, space="PSUM") as p1pool,
            tc.tile_pool(name="p2pool", bufs=2, space="PSUM") as p2pool,
        ):
            def load_w(name, dram, shape, dt_, round_to=None):
                t = wpool.tile(shape, dt_, tag=name)
                nc.sync.dma_start(t[:], dram[:])
                if round_to is None:
                    return t
                tr = wpool.tile(shape, round_to, tag=name + "r")
                nc.vector.tensor_copy(tr[:], t[:])
                return tr

            fdt = f16 if FEAT_F16 else f32r
            wm1_sb = load_w("wm1", wm1, [KX, RW], f16)
            w2q_r = load_w("w2q", w2q, [RW, OW], f32, fdt)
            w2s_r = load_w("w2s", w2s, [RW, OW], f32, fdt)
            if TAIL_GPB:
                wm1t_sb = load_w("wm1t", wm1t, [KXt, RWt], f16)
                w2qt_r = load_w("w2qt", w2qt, [RWt, OWt], f32, fdt)
                w2st_r = load_w("w2st", w2st, [RWt, OWt], f32, fdt)

            for _ in range(2):
                xt0 = xpool.tile([KX, ND], f16, tag="X")
                nc.gpsimd.memset(xt0[0:1, :], 1.0)

            # Software pipeline: stage1 (DMA-in + M1) of compute-chunk j is
            # emitted before stage2 (Square/cube/M2/copy-out + DMA-out) of
            # chunk j-1, so no engine queue head-of-line blocks on the
            # M1 -> SQ -> TT -> M2 dependency chain.
            asp = (int(NCMP * ACT_FRAC) // 2) * 2

            def stage1(lo, gpb, m1w, tagsuf, q, xt_state):
                kx = 3 * gpb * BPC + 1
                rw = R * gpb * BPC
                if q == 0:
                    xt = xpool.tile([kx, ND], f16, tag="X" + tagsuf, name="xt" + tagsuf)
                    if tagsuf:
                        nc.gpsimd.memset(xt[0:1, :], 1.0)
                    for b in range(BPC):
                        nc.sync.dma_start(
                            xt[1+b*3*gpb:1+(b+1)*3*gpb],
                            xs[b, :, lo:lo+gpb*ND].rearrange("v (g n) -> v g n", n=ND))
                    xt_state[tagsuf] = xt
                xt = xt_state[tagsuf]
                cl = q * NCMP
                p1 = p1pool.tile([rw, NCMP], f32, tag="P1")
                for h in range(NCMP // 512):
                    nc.tensor.matmul(p1[:, h*512:(h+1)*512], m1w[:],
                                     xt[:, cl+h*512:cl+(h+1)*512],
                                     start=True, stop=True)
                return p1

            def stage2(gpb, p1):
                rw = R * gpb * BPC
                s = spool.tile([rw, NCMP], f16 if FEAT_F16 else f32r, tag="S")
                nc.scalar.square(s[:], p1[:])
                qq = qpool.tile([rw, NCMP], f16 if FEAT_F16 else f32r, tag="Q")
                nc.vector.tensor_mul(qq[:], s[:], p1[:])
                return s, qq

            def stage3(lo, gpb, qw, sw, tagsuf, q, s, qq, o_state):
                rw = R * gpb * BPC
                ow = 3 * gpb * BPC
                if q == 0:
                    o_state[tagsuf] = opool.tile([ow, ND], f32, tag="O" + tagsuf, name="o" + tagsuf)
                o = o_state[tagsuf]
                cl, ch = q * NCMP, (q + 1) * NCMP
                p2 = p2pool.tile([ow, NCMP], f32, tag="P2")
                for h in range(NCMP // 512):
                    hl, hh = h*512, (h+1)*512
                    nc.tensor.matmul(p2[:, hl:hh], qw[:], qq[:, hl:hh],
                                     start=True, stop=False)
                for h in range(NCMP // 512):
                    hl, hh = h*512, (h+1)*512
                    nc.tensor.matmul(p2[:, hl:hh], sw[:], s[:, hl:hh],
                                     start=False, stop=True)
                nc.scalar.copy(o[:, cl:cl+asp], p2[:, 0:asp])
                nc.vector.tensor_copy(o[:, cl+asp:ch], p2[:, asp:NCMP])
                if q == SPLIT - 1:
                    for b in range(BPC):
                        nc.sync.dma_start(
                            y[b, :, lo:lo+gpb*ND].rearrange("c (g n) -> c g n", n=ND),
                            o[b*3*gpb:(b+1)*3*gpb])

            def body():
                nfull = FULL_CHUNKS if chunks is None else chunks
                work = [(k * GPB * ND, GPB, wm1_sb, w2q_r, w2s_r, "")
                        for k in range(nfull)]
                if TAIL_GPB and chunks is None:
                    work.append((FULL_CHUNKS * GPB * ND, TAIL_GPB,
                                 wm1t_sb, w2qt_r, w2st_r, "T"))
                units = [(lo, gpb, m1w, qw, sw, tagsuf, q)
                         for (lo, gpb, m1w, qw, sw, tagsuf) in work
                         for q in range(SPLIT)]
                xt_state, o_state = {}, {}
                q2 = []   # awaiting stage2, FIFO
                q3 = []   # awaiting stage3, FIFO
                D2, D3 = 1, 1   # stage separation depths (units)

                def pump(force=False):
                    if q2 and (force or len(q2) > D2 - 1):
                        (l2, g2, q2w, s2w, t2, qu2, p12) = q2.pop(0)
                        s_t, qq_t = stage2(g2, p12)
                        q3.append((l2, g2, q2w, s2w, t2, qu2, s_t, qq_t, o_state))
                    if q3 and (force or len(q3) > D3 - 1):
                        stage3(*q3.pop(0))

                for (lo, gpb, m1w, qw, sw, tagsuf, q) in units:
                    p1 = stage1(lo, gpb, m1w, tagsuf, q, xt_state)
                    q2.append((lo, gpb, qw, sw, tagsuf, q, p1))
                    pump()
                while q2 or q3:
                    pump(force=True)

            if reps == 1:
                body()
            else:
                hint = (mybir.EngineType.PE, mybir.EngineType.Activation,
                        mybir.EngineType.DVE, mybir.EngineType.SP)
                with tc.For_i(0, reps, 1, hint_engines=hint):
                    body()

    nc.compile()
    _NC_CACHE[key] = nc
    return nc


def make_in_maps(x, weight, bias):
    av, bv, cq, cs = _solve_coeffs(weight, bias)
    shared = {
        "wm1": _lhs1(av, bv, GPB),
        "w2q": _lhs2(cq, GPB), "w2s": _lhs2(cs, GPB),
    }
    if TAIL_GPB:
        shared.update({
            "wm1t": _lhs1(av, bv, TAIL_GPB),
            "w2qt": _lhs2(cq, TAIL_GPB), "w2st": _lhs2(cs, TAIL_GPB),
        })
    x = np.ascontiguousarray(np.asarray(x, np.float16)).reshape(B, C, HW)
    return [dict(shared, xs=x[i*BPC:(i+1)*BPC]) for i in range(NCORES)]


def kernel(x, weight, bias, degree=3, **_unused):
    assert int(degree) == 3, "kernel specialized for degree=3"
    nc = build_nc(reps=1)
    in_maps = make_in_maps(x, weight, bias)
    res = run_bass_kernel_spmd(nc, in_maps, core_ids=list(range(NCORES)))
    out = np.empty((B, C, HW), np.float32)
    for i in range(NCORES):
        out[i*BPC:(i+1)*BPC] = res.results[i]["y"]
    return out.reshape(B, C, H, W)


if __name__ == "__main__":
    rng = np.random.default_rng(0)
    x = rng.uniform(0, 1, size=(B, C, H, W)).astype(np.float32)
    weight = rng.normal(size=(19, 3)).astype(np.float32)
    bias = rng.normal(size=(3,)).astype(np.float32)
    got = kernel(x, weight, bias, 3)
    print("ran; out shape", got.shape)
